# revision 16
# baseline (speedup 1.0000x reference)
"""NeRF MLP forward pass on 8 Trainium2 NeuronCores (Bass/Tile).

Strategy: pure data parallel over rays. Each core processes 512 rays x 64
samples = 32768 points through the full MLP. Activations live transposed in
SBUF as [hidden, n_points] so every linear layer is a chain of
128x128 (stationary weight) x [128, 512] (moving activations) matmuls in
float32r (full-rate fp32 with 11-bit mantissa). Harmonic embeddings are
computed on-chip with Cody-Waite range reduction + the ScalarE Sin LUT.
"""

import sys

if '/opt/trn_rl_repo' not in sys.path:
    sys.path.insert(0, '/opt/trn_rl_repo')

import numpy as np

import concourse.bacc as bacc
import concourse.mybir as mybir
import concourse.tile as tile
from concourse.bass_utils import run_bass_kernel_spmd

F32 = mybir.dt.float32
F32R = mybir.dt.float32r
AF = mybir.ActivationFunctionType
ALU = mybir.AluOpType

N_CORES = 8
N_RAYS, S = 4096, 64
R_CORE = N_RAYS // N_CORES            # 512 rays per core
NPTS = R_CORE * S                     # 32768 points per core
# Points are ordered SAMPLE-major per core: point index = s * R_CORE + r.
# A 512-point sub-tile is then exactly one sample index across all rays,
# and the per-ray direction embedding broadcast is a contiguous block
# repeat.
F = 512                               # points per matmul (one PSUM bank)
FSUP = 2048                           # points per super-tile
NSUB = FSUP // F                      # 4
NSUP = NPTS // FSUP                   # 16
S_SUP = FSUP // R_CORE                # 4 samples per super-tile

H = 256
EMB_X = 63
EMB_D = 27

PI = float(np.pi)
INV2PI = float(1.0 / (2.0 * np.pi))
MAGIC = float(1.5 * 2 ** 23)
# Cody-Waite split of 2*pi: c1 exact in 9 mantissa bits, c2 in ~12, c3 rest.
_t = 2.0 * np.pi - 6.28125
_c2u = np.float32(_t).view(np.uint32) & np.uint32(0xFFFFF000)
CW1 = 6.28125
CW2 = float(_c2u.view(np.float32))
CW3 = float(np.float32(_t - float(_c2u.view(np.float32))))

# (name, kparts, out_chunks) ; kparts entries: (src, chunk_idx, ksize)
_XYZ_LAYERS = []
for li in range(8):
    if li == 0:
        kparts = [("E", 0, EMB_X)]
    elif li == 4:
        kparts = [("x", 0, 128), ("x", 1, 128), ("E", 0, EMB_X)]
    else:
        kparts = [("x", 0, 128), ("x", 1, 128)]
    _XYZ_LAYERS.append(kparts)

_cache = {}


def _build():
    if "nc" in _cache:
        return _cache["nc"]

    nc = bacc.Bacc("TRN2", target_bir_lowering=False, debug=False,
                   num_devices=N_CORES)

    pts = nc.dram_tensor("pts", [3, NPTS], F32, kind="ExternalInput")
    dirs = nc.dram_tensor("dirs", [3, R_CORE], F32, kind="ExternalInput")
    w0 = nc.dram_tensor("w0", [EMB_X, 256], F32, kind="ExternalInput")
    wmid = {i: nc.dram_tensor(f"wmid{i}", [128, 512], F32, kind="ExternalInput")
            for i in range(1, 8)}
    w4e = nc.dram_tensor("w4e", [EMB_X, 256], F32, kind="ExternalInput")
    wfeat = nc.dram_tensor("wfeat", [128, 512], F32, kind="ExternalInput")
    wden = nc.dram_tensor("wden", [128, 2], F32, kind="ExternalInput")
    wd0 = nc.dram_tensor("wd0", [128, 256], F32, kind="ExternalInput")
    wd0e = nc.dram_tensor("wd0e", [EMB_D, 128], F32, kind="ExternalInput")
    wrgb = nc.dram_tensor("wrgb", [128, 3], F32, kind="ExternalInput")
    biases = nc.dram_tensor("biases", [128, 21], F32, kind="ExternalInput")
    consts = nc.dram_tensor("consts", [128, 4], F32, kind="ExternalInput")
    out = nc.dram_tensor("out", [4, NPTS], F32, kind="ExternalOutput")

    with tile.TileContext(nc) as tc:
        with (
            tc.tile_pool(name="wpool", bufs=1) as wpool,
            tc.tile_pool(name="epool", bufs=2) as epool,
            tc.tile_pool(name="spool", bufs=1) as spool,
            tc.tile_pool(name="apool", bufs=1) as apool,
            tc.tile_pool(name="opool", bufs=2) as opool,
            tc.tile_pool(name="psum", bufs=8, space="PSUM") as psum,
        ):
            # ---- load weights / constants (once) ----
            w0_t = wpool.tile([EMB_X, 256], F32R)
            nc.sync.dma_start(w0_t[:], w0[:].bitcast(F32R))
            wmid_t = {}
            for i in range(1, 8):
                wt = wpool.tile([128, 512], F32R, name=f"wmid{i}_t")
                nc.sync.dma_start(wt[:], wmid[i][:].bitcast(F32R))
                wmid_t[i] = wt
            w4e_t = wpool.tile([EMB_X, 256], F32R)
            nc.sync.dma_start(w4e_t[:], w4e[:].bitcast(F32R))
            wfeat_t = wpool.tile([128, 512], F32R)
            nc.sync.dma_start(wfeat_t[:], wfeat[:].bitcast(F32R))
            wden_t = wpool.tile([128, 2], F32R)
            nc.sync.dma_start(wden_t[:], wden[:].bitcast(F32R))
            wd0_t = wpool.tile([128, 256], F32R)
            nc.sync.dma_start(wd0_t[:], wd0[:].bitcast(F32R))
            wd0e_t = wpool.tile([EMB_D, 128], F32R)
            nc.sync.dma_start(wd0e_t[:], wd0e[:].bitcast(F32R))
            wrgb_t = wpool.tile([128, 3], F32R)
            nc.sync.dma_start(wrgb_t[:], wrgb[:].bitcast(F32R))
            b_t = wpool.tile([128, 21], F32)
            nc.sync.dma_start(b_t[:], biases[:])
            c_t = wpool.tile([128, 4], F32)
            nc.sync.dma_start(c_t[:], consts[:])
            zeros_t = wpool.tile([128, 1], F32)
            nc.any.memset(zeros_t[:], 0.0)

            def sincos(dst, scratch_p, scratch_k, freqshift, nrows):
                """dst[0:2*nrows] = [sin(a), cos(a)] with a = raw args
                replicated in both halves of scratch_p. freqshift is a
                [2*nrows, 2] fp32 AP: col0 per-row freq scale, col1 per-row
                shift (pi/2 for the cos half). Scratch is destroyed."""
                nc.vector.tensor_scalar(scratch_p[:], scratch_p[:],
                                        freqshift[:, 0:1], freqshift[:, 1:2],
                                        op0=ALU.mult, op1=ALU.add)
                nc.vector.tensor_scalar(scratch_k[:], scratch_p[:], INV2PI,
                                        MAGIC, op0=ALU.mult, op1=ALU.add)
                nc.vector.tensor_scalar(scratch_k[:], scratch_k[:], MAGIC,
                                        None, op0=ALU.subtract)
                nc.vector.cody_waite_cascade(scratch_p[:], scratch_p[:],
                                             scratch_k[:], CW1, CW2, CW3)
                nc.scalar.activation(dst[0:2 * nrows, :], scratch_p[:],
                                     AF.Sin, bias=zeros_t[0:2 * nrows, 0:1])

            # ---- direction embedding per ray (once per core) ----
            embd_rays = wpool.tile([EMB_D, R_CORE], F32R)
            pd = wpool.tile([24, R_CORE], F32)
            kd = wpool.tile([24, R_CORE], F32)
            for half in range(2):
                for c in range(3):
                    nc.sync.dma_start(
                        pd[half * 12 + c * 4:half * 12 + (c + 1) * 4, :],
                        dirs[c:c + 1, :].partition_broadcast(4))
            sincos(embd_rays, pd, kd, c_t[0:24, 2:4], 12)
            nc.sync.dma_start(embd_rays[24:27, :], dirs[:].bitcast(F32R))

            # ---- per super-tile pipeline ----
            ev_counter = [0]

            def evict(psum_ap, out_ap, bias_ap):
                """relu(psum + bias) -> fp32r SBUF, alternating ACT/DVE."""
                if ev_counter[0] % 2 == 0:
                    nc.scalar.activation(out_ap, psum_ap, AF.Relu,
                                         bias=bias_ap)
                else:
                    nc.vector.tensor_scalar(out_ap, psum_ap, bias_ap, 0.0,
                                            op0=ALU.add, op1=ALU.max)
                ev_counter[0] += 1

            for st in range(NSUP):
                sl = slice(st * FSUP, (st + 1) * FSUP)

                # xyz harmonic embedding for this super-tile
                P = spool.tile([60, FSUP], F32, name="P")
                K = spool.tile([60, FSUP], F32, name="K")
                for half in range(2):
                    for c in range(3):
                        nc.sync.dma_start(
                            P[half * 30 + c * 10:half * 30 + (c + 1) * 10, :],
                            pts[c:c + 1, sl].partition_broadcast(10))
                E = epool.tile([EMB_X, FSUP], F32R, name="E")
                sincos(E, P, K, c_t[0:60, 0:2], 30)
                nc.sync.dma_start(E[60:63, :], pts[:, sl].bitcast(F32R))

                # broadcast direction embedding to per-point
                embd = epool.tile([EMB_D, FSUP], F32R, name="embd")
                nc.sync.dma_start(
                    embd[:].rearrange("p (s r) -> p s r", s=S_SUP),
                    embd_rays[:].unsqueeze(1)
                    .broadcast_to([EMB_D, S_SUP, R_CORE]))

                xa = apool.tile([128, 2 * FSUP], F32R, name="xa")
                xb = apool.tile([128, 2 * FSUP], F32R, name="xb")
                h = apool.tile([128, FSUP], F32R, name="h")
                osb = opool.tile([1, FSUP], F32, name="osb")
                rgbsb = opool.tile([3, FSUP], F32, name="rgbsb")

                def xsl(t, chunk, sub):
                    return t[:, chunk * FSUP + sub * F:
                             chunk * FSUP + sub * F + F]

                def rhs_of(src, idx, ksz, cur, sub):
                    if src == "E":
                        return E[0:EMB_X, sub * F:(sub + 1) * F]
                    if src == "embd":
                        return embd[0:EMB_D, sub * F:(sub + 1) * F]
                    return xsl(cur, idx, sub)

                cur = None
                # 8 xyz layers
                for li, kparts in enumerate(_XYZ_LAYERS):
                    nxt = xa if li % 2 == 0 else xb
                    for m in range(2):
                        lhs = []
                        for k, (src, idx, ksz) in enumerate(kparts):
                            if li == 0:
                                lt = w0_t[:, m * 128:(m + 1) * 128]
                            elif src == "E":
                                lt = w4e_t[:, m * 128:(m + 1) * 128]
                            else:
                                lt = wmid_t[li][:, idx * 256 + m * 128:
                                                idx * 256 + m * 128 + 128]
                            lhs.append(lt)
                        for sub in range(NSUB):
                            pt = psum.tile([128, F], F32, name="mmps",
                                           tag="mm")
                            for k, (src, idx, ksz) in enumerate(kparts):
                                nc.tensor.matmul(
                                    pt[:], lhs[k][0:ksz, :],
                                    rhs_of(src, idx, ksz, cur, sub),
                                    start=(k == 0),
                                    stop=(k == len(kparts) - 1))
                            evict(pt[:], xsl(nxt, m, sub),
                                  b_t[:, 2 * li + m:2 * li + m + 1])
                    cur = nxt

                # density head + feat layer (both read cur = x7)
                for sub in range(NSUB):
                    ptd = psum.tile([1, F], F32, name="denps", tag="mm")
                    for k in range(2):
                        nc.tensor.matmul(ptd[:], wden_t[:, k:k + 1],
                                         xsl(cur, k, sub),
                                         start=(k == 0), stop=(k == 1))
                    nc.scalar.activation(osb[0:1, sub * F:(sub + 1) * F],
                                         ptd[:], AF.Relu,
                                         bias=b_t[0:1, 19:20])
                nxt = xa if cur is xb else xb  # feat output
                for m in range(2):
                    for sub in range(NSUB):
                        pt = psum.tile([128, F], F32, name="featps", tag="mm")
                        for k in range(2):
                            nc.tensor.matmul(
                                pt[:],
                                wfeat_t[:, k * 256 + m * 128:
                                        k * 256 + m * 128 + 128],
                                xsl(cur, k, sub),
                                start=(k == 0), stop=(k == 1))
                        evict(pt[:], xsl(nxt, m, sub),
                              b_t[:, 16 + m:17 + m])
                cur = nxt

                # direction layer -> h
                for sub in range(NSUB):
                    pt = psum.tile([128, F], F32, name="dirps", tag="mm")
                    nc.tensor.matmul(pt[:], wd0_t[:, 0:128],
                                     xsl(cur, 0, sub), start=True, stop=False)
                    nc.tensor.matmul(pt[:], wd0_t[:, 128:256],
                                     xsl(cur, 1, sub), start=False, stop=False)
                    nc.tensor.matmul(pt[:], wd0e_t[:],
                                     embd[0:EMB_D, sub * F:(sub + 1) * F],
                                     start=False, stop=True)
                    evict(pt[:], h[:, sub * F:(sub + 1) * F],
                          b_t[:, 18:19])

                # rgb head
                for sub in range(NSUB):
                    ptr = psum.tile([3, F], F32, name="rgbps", tag="mm")
                    nc.tensor.matmul(ptr[:], wrgb_t[:],
                                     h[:, sub * F:(sub + 1) * F],
                                     start=True, stop=True)
                    nc.scalar.activation(rgbsb[:, sub * F:(sub + 1) * F],
                                         ptr[:], AF.Sigmoid,
                                         bias=b_t[0:3, 20:21])

                nc.sync.dma_start(out[0:1, sl], osb[:])
                nc.sync.dma_start(out[1:4, sl], rgbsb[:])

    nc.compile()
    _cache["nc"] = nc
    return nc


def _round_fp32r(a):
    """Round-to-nearest-even at 11 mantissa bits (matches HW fp32r input
    rounding) - applied host-side to weights for determinism."""
    u = np.ascontiguousarray(a, dtype=np.float32).view(np.uint32).astype(np.uint64)
    r = ((u + 0x800 + ((u >> 12) & 1)) >> 12 << 12).astype(np.uint32)
    return r.view(np.float32).reshape(a.shape)


def _prep_inputs(inputs):
    """Host-side shard + transpose prep. Returns list of per-core dicts."""
    f32 = np.float32
    sp = np.ascontiguousarray(inputs["sample_points"], dtype=f32)
    dirs = np.ascontiguousarray(inputs["directions"], dtype=f32)

    dirs_all = dirs.T.copy()                          # [3, 4096]

    def wt(w):  # [out, in] -> [in, out]
        return np.ascontiguousarray(w.T, dtype=f32)

    def wmid_pack(w):  # [256, 256] -> [128, 512] (k-chunk blocks)
        t = wt(w)                                     # [256, 256]
        return np.ascontiguousarray(
            t.reshape(2, 128, 256).transpose(1, 0, 2).reshape(128, 512))

    shared = {}
    shared["w0"] = wt(inputs["Wx0"])                  # [63, 256]
    for i in range(1, 8):
        w = inputs[f"Wx{i}"]
        if i == 4:
            shared["wmid4"] = wmid_pack(w[:, :256])
            shared["w4e"] = wt(w[:, 256:])            # [63, 256]
        else:
            shared[f"wmid{i}"] = wmid_pack(w)
    shared["wfeat"] = wmid_pack(inputs["Wfeat"])
    wden_t = wt(inputs["Wden"])                       # [256, 1]
    shared["wden"] = np.ascontiguousarray(
        wden_t.reshape(2, 128, 1).transpose(1, 0, 2).reshape(128, 2))
    wd0_t = wt(inputs["Wd0"])                         # [283, 128]
    shared["wd0"] = np.ascontiguousarray(
        wd0_t[:256].reshape(2, 128, 128).transpose(1, 0, 2).reshape(128, 256))
    shared["wd0e"] = np.ascontiguousarray(wd0_t[256:])  # [27, 128]
    shared["wrgb"] = wt(inputs["Wrgb"])               # [128, 3]

    bias = np.zeros((128, 21), dtype=f32)
    for li in range(8):
        b = inputs[f"bx{li}"]
        bias[:, 2 * li] = b[:128]
        bias[:, 2 * li + 1] = b[128:]
    bias[:, 16] = inputs["bfeat"][:128]
    bias[:, 17] = inputs["bfeat"][128:]
    bias[:, 18] = inputs["bd0"]
    bias[0, 19] = inputs["bden"][0]
    bias[0:3, 20] = inputs["brgb"]

    consts = np.zeros((128, 4), dtype=f32)
    consts[0:30, 0] = 2.0 ** (np.arange(30) % 10)
    consts[30:60, 0] = 2.0 ** (np.arange(30) % 10)
    consts[30:60, 1] = np.pi / 2
    consts[0:12, 2] = 2.0 ** (np.arange(12) % 4)
    consts[12:24, 2] = 2.0 ** (np.arange(12) % 4)
    consts[12:24, 3] = np.pi / 2

    in_maps = []
    for c in range(N_CORES):
        m = dict(shared)
        # sample-major: [3, S, R] flattened to [3, NPTS]
        blk = sp[c * R_CORE:(c + 1) * R_CORE]         # [R, S, 3]
        m["pts"] = np.ascontiguousarray(
            blk.transpose(2, 1, 0).reshape(3, NPTS))
        m["dirs"] = np.ascontiguousarray(
            dirs_all[:, c * R_CORE:(c + 1) * R_CORE])
        m["biases"] = bias
        m["consts"] = consts
        in_maps.append(m)
    return in_maps


def kernel(**inputs) -> np.ndarray:
    nc = _build()
    in_maps = _prep_inputs(inputs)
    res = run_bass_kernel_spmd(nc, in_maps, core_ids=list(range(N_CORES)))
    outs = []
    for c in range(N_CORES):
        o = res.results[c]["out"]                     # [4, NPTS] sample-major
        outs.append(o.reshape(4, S, R_CORE).transpose(2, 1, 0))
    return np.concatenate(outs, axis=0)


# revision 20
# speedup vs baseline: 1683.1940x; 1683.1940x over previous
"""NeRF MLP forward pass on 8 Trainium2 NeuronCores (Bass/Tile).

Strategy: pure data parallel over rays. Each core processes 512 rays x 64
samples = 32768 points through the full MLP. Activations live transposed in
SBUF as [hidden, n_points] so every linear layer is a chain of
128x128 (stationary weight) x [128, 512] (moving activations) matmuls in
float32r (full-rate fp32 with 11-bit mantissa). Harmonic embeddings are
computed on-chip with Cody-Waite range reduction + the ScalarE Sin LUT.
"""

import sys

if '/opt/trn_rl_repo' not in sys.path:
    sys.path.insert(0, '/opt/trn_rl_repo')

import numpy as np

import concourse.bacc as bacc
import concourse.mybir as mybir
import concourse.tile as tile
from concourse.bass_utils import run_bass_kernel_spmd

F32 = mybir.dt.float32
F32R = mybir.dt.float32r
AF = mybir.ActivationFunctionType
ALU = mybir.AluOpType

N_CORES = 8
N_RAYS, S = 4096, 64
R_CORE = N_RAYS // N_CORES            # 512 rays per core
NPTS = R_CORE * S                     # 32768 points per core
# Points are ordered SAMPLE-major per core: point index = s * R_CORE + r.
# A 512-point sub-tile is then exactly one sample index across all rays,
# and the per-ray direction embedding broadcast is a contiguous block
# repeat.
F = 512                               # points per matmul (one PSUM bank)
FSUP = 2048                           # points per super-tile
NSUB = FSUP // F                      # 4
NSUP = NPTS // FSUP                   # 16
S_SUP = FSUP // R_CORE                # 4 samples per super-tile

H = 256
EMB_X = 63
EMB_D = 27

PI = float(np.pi)
INV2PI = float(1.0 / (2.0 * np.pi))
MAGIC = float(1.5 * 2 ** 23)
# Cody-Waite split of 2*pi: c1 exact in 9 mantissa bits, c2 in ~12, c3 rest.
_t = 2.0 * np.pi - 6.28125
_c2u = np.float32(_t).view(np.uint32) & np.uint32(0xFFFFF000)
CW1 = 6.28125
CW2 = float(_c2u.view(np.float32))
CW3 = float(np.float32(_t - float(_c2u.view(np.float32))))

# (name, kparts, out_chunks) ; kparts entries: (src, chunk_idx, ksize)
_XYZ_LAYERS = []
for li in range(8):
    if li == 0:
        kparts = [("E", 0, EMB_X)]
    elif li == 4:
        kparts = [("x", 0, 128), ("x", 1, 128), ("E", 0, EMB_X)]
    else:
        kparts = [("x", 0, 128), ("x", 1, 128)]
    _XYZ_LAYERS.append(kparts)

_cache = {}


def _build(nsup_exec=NSUP):
    """Build the bass program. nsup_exec > NSUP repeats super-tiles
    (st = i % NSUP) — used only for slope-based timing benchmarks."""
    key = ("nc", nsup_exec)
    if key in _cache:
        return _cache[key]

    nc = bacc.Bacc("TRN2", target_bir_lowering=False, debug=False,
                   num_devices=N_CORES)

    pts = nc.dram_tensor("pts", [3, NPTS], F32, kind="ExternalInput")
    dirs = nc.dram_tensor("dirs", [3, R_CORE], F32, kind="ExternalInput")
    w0 = nc.dram_tensor("w0", [EMB_X, 256], F32, kind="ExternalInput")
    wmid = {i: nc.dram_tensor(f"wmid{i}", [128, 512], F32, kind="ExternalInput")
            for i in range(1, 8)}
    w4e = nc.dram_tensor("w4e", [EMB_X, 256], F32, kind="ExternalInput")
    wfeat = nc.dram_tensor("wfeat", [128, 512], F32, kind="ExternalInput")
    wden = nc.dram_tensor("wden", [128, 2], F32, kind="ExternalInput")
    wd0 = nc.dram_tensor("wd0", [128, 256], F32, kind="ExternalInput")
    wd0e = nc.dram_tensor("wd0e", [EMB_D, 128], F32, kind="ExternalInput")
    wrgb = nc.dram_tensor("wrgb", [128, 3], F32, kind="ExternalInput")
    biases = nc.dram_tensor("biases", [128, 21], F32, kind="ExternalInput")
    consts = nc.dram_tensor("consts", [128, 4], F32, kind="ExternalInput")
    out = nc.dram_tensor("out", [4, NPTS], F32, kind="ExternalOutput")

    with tile.TileContext(nc) as tc:
        with (
            tc.tile_pool(name="wpool", bufs=1) as wpool,
            tc.tile_pool(name="epool", bufs=2) as epool,
            tc.tile_pool(name="spool", bufs=1) as spool,
            tc.tile_pool(name="apool", bufs=1) as apool,
            tc.tile_pool(name="opool", bufs=2) as opool,
            tc.tile_pool(name="psum", bufs=8, space="PSUM") as psum,
        ):
            # ---- load weights / constants (once) ----
            w0_t = wpool.tile([EMB_X, 256], F32R)
            nc.sync.dma_start(w0_t[:], w0[:].bitcast(F32R))
            wmid_t = {}
            for i in range(1, 8):
                wt = wpool.tile([128, 512], F32R, name=f"wmid{i}_t")
                nc.sync.dma_start(wt[:], wmid[i][:].bitcast(F32R))
                wmid_t[i] = wt
            w4e_t = wpool.tile([EMB_X, 256], F32R)
            nc.sync.dma_start(w4e_t[:], w4e[:].bitcast(F32R))
            wfeat_t = wpool.tile([128, 512], F32R)
            nc.sync.dma_start(wfeat_t[:], wfeat[:].bitcast(F32R))
            wden_t = wpool.tile([128, 2], F32R)
            nc.sync.dma_start(wden_t[:], wden[:].bitcast(F32R))
            wd0_t = wpool.tile([128, 256], F32R)
            nc.sync.dma_start(wd0_t[:], wd0[:].bitcast(F32R))
            wd0e_t = wpool.tile([EMB_D, 128], F32R)
            nc.sync.dma_start(wd0e_t[:], wd0e[:].bitcast(F32R))
            wrgb_t = wpool.tile([128, 3], F32R)
            nc.sync.dma_start(wrgb_t[:], wrgb[:].bitcast(F32R))
            b_t = wpool.tile([128, 21], F32)
            nc.sync.dma_start(b_t[:], biases[:])
            c_t = wpool.tile([128, 4], F32)
            nc.sync.dma_start(c_t[:], consts[:])
            zeros_t = wpool.tile([128, 1], F32)
            nc.any.memset(zeros_t[:], 0.0)

            def sincos(dst, scratch_p, scratch_k, freqshift, nrows):
                """dst[0:2*nrows] = [sin(a), cos(a)] with a = raw args
                replicated in both halves of scratch_p. freqshift is a
                [2*nrows, 2] fp32 AP: col0 per-row freq scale, col1 per-row
                shift (pi/2 for the cos half). Scratch is destroyed."""
                nc.vector.tensor_scalar(scratch_p[:], scratch_p[:],
                                        freqshift[:, 0:1], freqshift[:, 1:2],
                                        op0=ALU.mult, op1=ALU.add)
                nc.vector.tensor_scalar(scratch_k[:], scratch_p[:], INV2PI,
                                        MAGIC, op0=ALU.mult, op1=ALU.add)
                nc.vector.tensor_scalar(scratch_k[:], scratch_k[:], MAGIC,
                                        None, op0=ALU.subtract)
                nc.vector.cody_waite_cascade(scratch_p[:], scratch_p[:],
                                             scratch_k[:], CW1, CW2, CW3)
                nc.scalar.activation(dst[0:2 * nrows, :], scratch_p[:],
                                     AF.Sin, bias=zeros_t[0:2 * nrows, 0:1])

            # ---- direction embedding per ray (once per core) ----
            embd_rays = wpool.tile([EMB_D, R_CORE], F32R)
            pd = wpool.tile([24, R_CORE], F32)
            kd = wpool.tile([24, R_CORE], F32)
            for half in range(2):
                for c in range(3):
                    nc.sync.dma_start(
                        pd[half * 12 + c * 4:half * 12 + (c + 1) * 4, :],
                        dirs[c:c + 1, :].partition_broadcast(4))
            sincos(embd_rays, pd, kd, c_t[0:24, 2:4], 12)
            nc.sync.dma_start(embd_rays[24:27, :], dirs[:].bitcast(F32R))

            # ---- per super-tile pipeline ----
            ev_counter = [0]

            def evict(psum_ap, out_ap, bias_ap):
                """relu(psum + bias) -> fp32r SBUF, alternating ACT/DVE."""
                if ev_counter[0] % 2 == 0:
                    nc.scalar.activation(out_ap, psum_ap, AF.Relu,
                                         bias=bias_ap)
                else:
                    nc.vector.tensor_scalar(out_ap, psum_ap, bias_ap, 0.0,
                                            op0=ALU.add, op1=ALU.max)
                ev_counter[0] += 1

            for sti in range(nsup_exec):
                st = sti % NSUP
                sl = slice(st * FSUP, (st + 1) * FSUP)

                # xyz harmonic embedding for this super-tile
                P = spool.tile([60, FSUP], F32, name="P")
                K = spool.tile([60, FSUP], F32, name="K")
                for half in range(2):
                    for c in range(3):
                        nc.sync.dma_start(
                            P[half * 30 + c * 10:half * 30 + (c + 1) * 10, :],
                            pts[c:c + 1, sl].partition_broadcast(10))
                E = epool.tile([EMB_X, FSUP], F32R, name="E")
                sincos(E, P, K, c_t[0:60, 0:2], 30)
                nc.sync.dma_start(E[60:63, :], pts[:, sl].bitcast(F32R))

                # broadcast direction embedding to per-point
                embd = epool.tile([EMB_D, FSUP], F32R, name="embd")
                nc.sync.dma_start(
                    embd[:].rearrange("p (s r) -> p s r", s=S_SUP),
                    embd_rays[:].unsqueeze(1)
                    .broadcast_to([EMB_D, S_SUP, R_CORE]))

                xa = apool.tile([128, 2 * FSUP], F32R, name="xa")
                xb = apool.tile([128, 2 * FSUP], F32R, name="xb")
                h = apool.tile([128, FSUP], F32R, name="h")
                osb = opool.tile([1, FSUP], F32, name="osb")
                rgbsb = opool.tile([3, FSUP], F32, name="rgbsb")

                def xsl(t, chunk, sub):
                    return t[:, chunk * FSUP + sub * F:
                             chunk * FSUP + sub * F + F]

                def rhs_of(src, idx, ksz, cur, sub):
                    if src == "E":
                        return E[0:EMB_X, sub * F:(sub + 1) * F]
                    if src == "embd":
                        return embd[0:EMB_D, sub * F:(sub + 1) * F]
                    return xsl(cur, idx, sub)

                cur = None
                # 8 xyz layers
                for li, kparts in enumerate(_XYZ_LAYERS):
                    nxt = xa if li % 2 == 0 else xb
                    for m in range(2):
                        lhs = []
                        for k, (src, idx, ksz) in enumerate(kparts):
                            if li == 0:
                                lt = w0_t[:, m * 128:(m + 1) * 128]
                            elif src == "E":
                                lt = w4e_t[:, m * 128:(m + 1) * 128]
                            else:
                                lt = wmid_t[li][:, idx * 256 + m * 128:
                                                idx * 256 + m * 128 + 128]
                            lhs.append(lt)
                        for sub in range(NSUB):
                            pt = psum.tile([128, F], F32, name="mmps",
                                           tag="mm")
                            for k, (src, idx, ksz) in enumerate(kparts):
                                nc.tensor.matmul(
                                    pt[:], lhs[k][0:ksz, :],
                                    rhs_of(src, idx, ksz, cur, sub),
                                    start=(k == 0),
                                    stop=(k == len(kparts) - 1))
                            evict(pt[:], xsl(nxt, m, sub),
                                  b_t[:, 2 * li + m:2 * li + m + 1])
                    cur = nxt

                # density head + feat layer (both read cur = x7)
                for sub in range(NSUB):
                    ptd = psum.tile([1, F], F32, name="denps", tag="mm")
                    for k in range(2):
                        nc.tensor.matmul(ptd[:], wden_t[:, k:k + 1],
                                         xsl(cur, k, sub),
                                         start=(k == 0), stop=(k == 1))
                    nc.scalar.activation(osb[0:1, sub * F:(sub + 1) * F],
                                         ptd[:], AF.Relu,
                                         bias=b_t[0:1, 19:20])
                nxt = xa if cur is xb else xb  # feat output
                for m in range(2):
                    for sub in range(NSUB):
                        pt = psum.tile([128, F], F32, name="featps", tag="mm")
                        for k in range(2):
                            nc.tensor.matmul(
                                pt[:],
                                wfeat_t[:, k * 256 + m * 128:
                                        k * 256 + m * 128 + 128],
                                xsl(cur, k, sub),
                                start=(k == 0), stop=(k == 1))
                        evict(pt[:], xsl(nxt, m, sub),
                              b_t[:, 16 + m:17 + m])
                cur = nxt

                # direction layer -> h
                for sub in range(NSUB):
                    pt = psum.tile([128, F], F32, name="dirps", tag="mm")
                    nc.tensor.matmul(pt[:], wd0_t[:, 0:128],
                                     xsl(cur, 0, sub), start=True, stop=False)
                    nc.tensor.matmul(pt[:], wd0_t[:, 128:256],
                                     xsl(cur, 1, sub), start=False, stop=False)
                    nc.tensor.matmul(pt[:], wd0e_t[:],
                                     embd[0:EMB_D, sub * F:(sub + 1) * F],
                                     start=False, stop=True)
                    evict(pt[:], h[:, sub * F:(sub + 1) * F],
                          b_t[:, 18:19])

                # rgb head
                for sub in range(NSUB):
                    ptr = psum.tile([3, F], F32, name="rgbps", tag="mm")
                    nc.tensor.matmul(ptr[:], wrgb_t[:],
                                     h[:, sub * F:(sub + 1) * F],
                                     start=True, stop=True)
                    nc.scalar.activation(rgbsb[:, sub * F:(sub + 1) * F],
                                         ptr[:], AF.Sigmoid,
                                         bias=b_t[0:3, 20:21])

                nc.sync.dma_start(out[0:1, sl], osb[:])
                nc.sync.dma_start(out[1:4, sl], rgbsb[:])

    nc.compile()
    _cache[key] = nc
    return nc


def _prep_inputs(inputs):
    """Host-side shard + transpose prep. Returns list of per-core dicts."""
    f32 = np.float32
    sp = np.ascontiguousarray(inputs["sample_points"], dtype=f32)
    dirs = np.ascontiguousarray(inputs["directions"], dtype=f32)

    dirs_all = dirs.T.copy()                          # [3, 4096]

    def wt(w):  # [out, in] -> [in, out]
        return np.ascontiguousarray(w.T, dtype=f32)

    def wmid_pack(w):  # [256, 256] -> [128, 512] (k-chunk blocks)
        t = wt(w)                                     # [256, 256]
        return np.ascontiguousarray(
            t.reshape(2, 128, 256).transpose(1, 0, 2).reshape(128, 512))

    shared = {}
    shared["w0"] = wt(inputs["Wx0"])                  # [63, 256]
    for i in range(1, 8):
        w = inputs[f"Wx{i}"]
        if i == 4:
            shared["wmid4"] = wmid_pack(w[:, :256])
            shared["w4e"] = wt(w[:, 256:])            # [63, 256]
        else:
            shared[f"wmid{i}"] = wmid_pack(w)
    shared["wfeat"] = wmid_pack(inputs["Wfeat"])
    wden_t = wt(inputs["Wden"])                       # [256, 1]
    shared["wden"] = np.ascontiguousarray(
        wden_t.reshape(2, 128, 1).transpose(1, 0, 2).reshape(128, 2))
    wd0_t = wt(inputs["Wd0"])                         # [283, 128]
    shared["wd0"] = np.ascontiguousarray(
        wd0_t[:256].reshape(2, 128, 128).transpose(1, 0, 2).reshape(128, 256))
    shared["wd0e"] = np.ascontiguousarray(wd0_t[256:])  # [27, 128]
    shared["wrgb"] = wt(inputs["Wrgb"])               # [128, 3]

    bias = np.zeros((128, 21), dtype=f32)
    for li in range(8):
        b = inputs[f"bx{li}"]
        bias[:, 2 * li] = b[:128]
        bias[:, 2 * li + 1] = b[128:]
    bias[:, 16] = inputs["bfeat"][:128]
    bias[:, 17] = inputs["bfeat"][128:]
    bias[:, 18] = inputs["bd0"]
    bias[0, 19] = inputs["bden"][0]
    bias[0:3, 20] = inputs["brgb"]

    consts = np.zeros((128, 4), dtype=f32)
    consts[0:30, 0] = 2.0 ** (np.arange(30) % 10)
    consts[30:60, 0] = 2.0 ** (np.arange(30) % 10)
    consts[30:60, 1] = np.pi / 2
    consts[0:12, 2] = 2.0 ** (np.arange(12) % 4)
    consts[12:24, 2] = 2.0 ** (np.arange(12) % 4)
    consts[12:24, 3] = np.pi / 2

    in_maps = []
    for c in range(N_CORES):
        m = dict(shared)
        # sample-major: [3, S, R] flattened to [3, NPTS]
        blk = sp[c * R_CORE:(c + 1) * R_CORE]         # [R, S, 3]
        m["pts"] = np.ascontiguousarray(
            blk.transpose(2, 1, 0).reshape(3, NPTS))
        m["dirs"] = np.ascontiguousarray(
            dirs_all[:, c * R_CORE:(c + 1) * R_CORE])
        m["biases"] = bias
        m["consts"] = consts
        in_maps.append(m)
    return in_maps


def kernel(**inputs) -> np.ndarray:
    nc = _build()
    in_maps = _prep_inputs(inputs)
    res = run_bass_kernel_spmd(nc, in_maps, core_ids=list(range(N_CORES)))
    outs = []
    for c in range(N_CORES):
        o = res.results[c]["out"]                     # [4, NPTS] sample-major
        outs.append(o.reshape(4, S, R_CORE).transpose(2, 1, 0))
    return np.concatenate(outs, axis=0)


# revision 34
# speedup vs baseline: 1730.2941x; 1.0280x over previous
"""NeRF MLP forward pass on 8 Trainium2 NeuronCores (Bass/Tile).

Strategy: pure data parallel over rays. Each core processes 512 rays x 64
samples = 32768 points through the full MLP. Activations live transposed in
SBUF as [hidden, n_points] so every linear layer is a chain of
128x128 (stationary weight) x [128, 512] (moving activations) matmuls in
float32r (full-rate fp32 with 11-bit mantissa). Harmonic embeddings are
computed on-chip with Cody-Waite range reduction + the ScalarE Sin LUT.
"""

import sys

if '/opt/trn_rl_repo' not in sys.path:
    sys.path.insert(0, '/opt/trn_rl_repo')

import numpy as np

import concourse.bacc as bacc
import concourse.mybir as mybir
import concourse.tile as tile
from concourse.bass_utils import run_bass_kernel_spmd

F32 = mybir.dt.float32
F32R = mybir.dt.float32r
AF = mybir.ActivationFunctionType
ALU = mybir.AluOpType

N_CORES = 8
N_RAYS, S = 4096, 64
R_CORE = N_RAYS // N_CORES            # 512 rays per core
NPTS = R_CORE * S                     # 32768 points per core
# Points are ordered SAMPLE-major per core: point index = s * R_CORE + r.
# A 512-point sub-tile is then exactly one sample index across all rays,
# and the per-ray direction embedding broadcast is a contiguous block
# repeat.
F = 512                               # points per matmul (one PSUM bank)
FSUP = 2048                           # points per super-tile
NSUB = FSUP // F                      # 4
NSUP = NPTS // FSUP                   # 16
S_SUP = FSUP // R_CORE                # 4 samples per super-tile

H = 256
EMB_X = 63
EMB_D = 27

PI = float(np.pi)
INV2PI = float(1.0 / (2.0 * np.pi))
MAGIC = float(1.5 * 2 ** 23)
# Cody-Waite split of 2*pi: c1 exact in 9 mantissa bits, c2 in ~12, c3 rest.
_t = 2.0 * np.pi - 6.28125
_c2u = np.float32(_t).view(np.uint32) & np.uint32(0xFFFFF000)
CW1 = 6.28125
CW2 = float(_c2u.view(np.float32))
CW3 = float(np.float32(_t - float(_c2u.view(np.float32))))

# (name, kparts, out_chunks) ; kparts entries: (src, chunk_idx, ksize)
_XYZ_LAYERS = []
for li in range(8):
    if li == 0:
        kparts = [("E", 0, EMB_X)]
    elif li == 4:
        kparts = [("x", 0, 128), ("x", 1, 128), ("E", 0, EMB_X)]
    else:
        kparts = [("x", 0, 128), ("x", 1, 128)]
    _XYZ_LAYERS.append(kparts)

_cache = {}


def _build(nsup_exec=NSUP):
    """Build the bass program. nsup_exec > NSUP repeats super-tiles
    (st = i % NSUP) — used only for slope-based timing benchmarks."""
    key = ("nc", nsup_exec)
    if key in _cache:
        return _cache[key]

    nc = bacc.Bacc("TRN2", target_bir_lowering=False, debug=False,
                   num_devices=N_CORES)

    pts = nc.dram_tensor("pts", [3, NPTS], F32, kind="ExternalInput")
    dirs = nc.dram_tensor("dirs", [3, R_CORE], F32, kind="ExternalInput")
    w0 = nc.dram_tensor("w0", [EMB_X, 256], F32, kind="ExternalInput")
    wmid = {i: nc.dram_tensor(f"wmid{i}", [128, 512], F32, kind="ExternalInput")
            for i in range(1, 8)}
    w4e = nc.dram_tensor("w4e", [EMB_X, 256], F32, kind="ExternalInput")
    wfeat = nc.dram_tensor("wfeat", [128, 512], F32, kind="ExternalInput")
    wden = nc.dram_tensor("wden", [128, 2], F32, kind="ExternalInput")
    wd0 = nc.dram_tensor("wd0", [128, 256], F32, kind="ExternalInput")
    wd0e = nc.dram_tensor("wd0e", [EMB_D, 128], F32, kind="ExternalInput")
    wrgb = nc.dram_tensor("wrgb", [128, 3], F32, kind="ExternalInput")
    biases = nc.dram_tensor("biases", [128, 21], F32, kind="ExternalInput")
    consts = nc.dram_tensor("consts", [128, 4], F32, kind="ExternalInput")
    out = nc.dram_tensor("out", [4, NPTS], F32, kind="ExternalOutput")

    with tile.TileContext(nc) as tc:
        with (
            tc.tile_pool(name="wpool", bufs=1) as wpool,
            tc.tile_pool(name="epool", bufs=3) as epool,
            tc.tile_pool(name="spool", bufs=2) as spool,
            tc.tile_pool(name="apool", bufs=1) as apool,
            tc.tile_pool(name="opool", bufs=2) as opool,
            tc.tile_pool(name="psum", bufs=8, space="PSUM") as psum,
        ):
            # ---- load weights / constants (once) ----
            w0_t = wpool.tile([EMB_X, 256], F32R)
            nc.sync.dma_start(w0_t[:], w0[:].bitcast(F32R))
            wmid_t = {}
            for i in range(1, 8):
                wt = wpool.tile([128, 512], F32R, name=f"wmid{i}_t")
                nc.sync.dma_start(wt[:], wmid[i][:].bitcast(F32R))
                wmid_t[i] = wt
            w4e_t = wpool.tile([EMB_X, 256], F32R)
            nc.sync.dma_start(w4e_t[:], w4e[:].bitcast(F32R))
            wfeat_t = wpool.tile([128, 512], F32R)
            nc.sync.dma_start(wfeat_t[:], wfeat[:].bitcast(F32R))
            wden_t = wpool.tile([128, 2], F32R)
            nc.sync.dma_start(wden_t[:], wden[:].bitcast(F32R))
            wd0_t = wpool.tile([128, 256], F32R)
            nc.sync.dma_start(wd0_t[:], wd0[:].bitcast(F32R))
            wd0e_t = wpool.tile([EMB_D, 128], F32R)
            nc.sync.dma_start(wd0e_t[:], wd0e[:].bitcast(F32R))
            wrgb_t = wpool.tile([128, 3], F32R)
            nc.sync.dma_start(wrgb_t[:], wrgb[:].bitcast(F32R))
            b_t = wpool.tile([128, 21], F32)
            nc.sync.dma_start(b_t[:], biases[:])
            c_t = wpool.tile([128, 4], F32)
            nc.sync.dma_start(c_t[:], consts[:])
            zeros_t = wpool.tile([128, 1], F32)
            nc.any.memset(zeros_t[:], 0.0)

            def sincos(dst, scratch_p, scratch_k, freqshift, nrows):
                """dst[0:2*nrows] = [sin(a), cos(a)] with a = raw args
                replicated in both halves of scratch_p. freqshift is a
                [2*nrows, 2] fp32 AP: col0 per-row freq scale, col1 per-row
                shift (pi/2 for the cos half). Scratch is destroyed."""
                nc.vector.tensor_scalar(scratch_p[:], scratch_p[:],
                                        freqshift[:, 0:1], freqshift[:, 1:2],
                                        op0=ALU.mult, op1=ALU.add)
                nc.vector.tensor_scalar(scratch_k[:], scratch_p[:], INV2PI,
                                        MAGIC, op0=ALU.mult, op1=ALU.add)
                nc.vector.tensor_scalar(scratch_k[:], scratch_k[:], MAGIC,
                                        None, op0=ALU.subtract)
                nc.vector.cody_waite_cascade(scratch_p[:], scratch_p[:],
                                             scratch_k[:], CW1, CW2, CW3)
                nc.scalar.activation(dst[0:2 * nrows, :], scratch_p[:],
                                     AF.Sin, bias=zeros_t[0:2 * nrows, 0:1])

            # ---- direction embedding per ray (once per core) ----
            embd_rays = wpool.tile([EMB_D, R_CORE], F32R)
            pd = wpool.tile([24, R_CORE], F32)
            kd = wpool.tile([24, R_CORE], F32)
            for half in range(2):
                for c in range(3):
                    nc.sync.dma_start(
                        pd[half * 12 + c * 4:half * 12 + (c + 1) * 4, :],
                        dirs[c:c + 1, :].partition_broadcast(4))
            sincos(embd_rays, pd, kd, c_t[0:24, 2:4], 12)
            nc.sync.dma_start(embd_rays[24:27, :], dirs[:].bitcast(F32R))

            # ---- per super-tile pipeline ----
            ev_counter = [0]

            def evict(psum_ap, out_ap, bias_ap):
                """relu(psum + bias) -> fp32r SBUF, alternating ACT/DVE."""
                if ev_counter[0] % 2 == 0:
                    nc.scalar.activation(out_ap, psum_ap, AF.Relu,
                                         bias=bias_ap)
                else:
                    nc.vector.tensor_scalar(out_ap, psum_ap, bias_ap, 0.0,
                                            op0=ALU.add, op1=ALU.max)
                ev_counter[0] += 1

            def emb_build(st):
                """Emit harmonic-embedding computation for super-tile st.
                Returns (E, embd) SBUF tiles."""
                sl = slice(st * FSUP, (st + 1) * FSUP)
                P = spool.tile([60, FSUP], F32, name="P")
                K = spool.tile([60, FSUP], F32, name="K")
                for half in range(2):
                    for c in range(3):
                        nc.sync.dma_start(
                            P[half * 30 + c * 10:half * 30 + (c + 1) * 10, :],
                            pts[c:c + 1, sl].partition_broadcast(10))
                E = epool.tile([EMB_X, FSUP], F32R, name="E")
                sincos(E, P, K, c_t[0:60, 0:2], 30)
                nc.sync.dma_start(E[60:63, :], pts[:, sl].bitcast(F32R))
                # broadcast direction embedding to per-point
                embd = epool.tile([EMB_D, FSUP], F32R, name="embd")
                nc.sync.dma_start(
                    embd[:].rearrange("p (s r) -> p s r", s=S_SUP),
                    embd_rays[:].unsqueeze(1)
                    .broadcast_to([EMB_D, S_SUP, R_CORE]))
                return E, embd

            emb_next = emb_build(0)
            for sti in range(nsup_exec):
                st = sti % NSUP
                sl = slice(st * FSUP, (st + 1) * FSUP)
                E, embd = emb_next

                xa = apool.tile([128, 2 * FSUP], F32R, name="xa")
                xb = apool.tile([128, 2 * FSUP], F32R, name="xb")
                h = apool.tile([128, FSUP], F32R, name="h")
                osb = opool.tile([1, FSUP], F32, name="osb")
                rgbsb = opool.tile([3, FSUP], F32, name="rgbsb")

                def xsl(t, chunk, sub):
                    return t[:, chunk * FSUP + sub * F:
                             chunk * FSUP + sub * F + F]

                def rhs_of(src, idx, ksz, cur, sub):
                    if src == "E":
                        return E[0:EMB_X, sub * F:(sub + 1) * F]
                    if src == "embd":
                        return embd[0:EMB_D, sub * F:(sub + 1) * F]
                    return xsl(cur, idx, sub)

                cur = None
                # 8 xyz layers
                for li, kparts in enumerate(_XYZ_LAYERS):
                    nxt = xa if li % 2 == 0 else xb
                    for sub in range(NSUB):
                        for m in range(2):
                            lhs = []
                            for k, (src, idx, ksz) in enumerate(kparts):
                                if li == 0:
                                    lt = w0_t[:, m * 128:(m + 1) * 128]
                                elif src == "E":
                                    lt = w4e_t[:, m * 128:(m + 1) * 128]
                                else:
                                    lt = wmid_t[li][:, idx * 256 + m * 128:
                                                    idx * 256 + m * 128 + 128]
                                lhs.append(lt)
                            pt = psum.tile([128, F], F32, name="mmps",
                                           tag="mm")
                            for k, (src, idx, ksz) in enumerate(kparts):
                                nc.tensor.matmul(
                                    pt[:], lhs[k][0:ksz, :],
                                    rhs_of(src, idx, ksz, cur, sub),
                                    start=(k == 0),
                                    stop=(k == len(kparts) - 1))
                            evict(pt[:], xsl(nxt, m, sub),
                                  b_t[:, 2 * li + m:2 * li + m + 1])
                    cur = nxt
                    if li == 5 and sti + 1 < nsup_exec:
                        # emit next super-tile's embedding mid-stream so the
                        # in-order ACT/DVE queues produce it before this
                        # super-tile's tail, keeping PE fed at the boundary
                        emb_next = emb_build((sti + 1) % NSUP)

                # density head + feat layer (both read cur = x7)
                for sub in range(NSUB):
                    ptd = psum.tile([1, F], F32, name="denps", tag="mm")
                    for k in range(2):
                        nc.tensor.matmul(ptd[:], wden_t[:, k:k + 1],
                                         xsl(cur, k, sub),
                                         start=(k == 0), stop=(k == 1))
                    nc.scalar.activation(osb[0:1, sub * F:(sub + 1) * F],
                                         ptd[:], AF.Relu,
                                         bias=b_t[0:1, 19:20])
                nxt = xa if cur is xb else xb  # feat output
                for sub in range(NSUB):
                    for m in range(2):
                        pt = psum.tile([128, F], F32, name="featps", tag="mm")
                        for k in range(2):
                            nc.tensor.matmul(
                                pt[:],
                                wfeat_t[:, k * 256 + m * 128:
                                        k * 256 + m * 128 + 128],
                                xsl(cur, k, sub),
                                start=(k == 0), stop=(k == 1))
                        evict(pt[:], xsl(nxt, m, sub),
                              b_t[:, 16 + m:17 + m])
                cur = nxt

                # direction layer -> h
                for sub in range(NSUB):
                    pt = psum.tile([128, F], F32, name="dirps", tag="mm")
                    nc.tensor.matmul(pt[:], wd0_t[:, 0:128],
                                     xsl(cur, 0, sub), start=True, stop=False)
                    nc.tensor.matmul(pt[:], wd0_t[:, 128:256],
                                     xsl(cur, 1, sub), start=False, stop=False)
                    nc.tensor.matmul(pt[:], wd0e_t[:],
                                     embd[0:EMB_D, sub * F:(sub + 1) * F],
                                     start=False, stop=True)
                    evict(pt[:], h[:, sub * F:(sub + 1) * F],
                          b_t[:, 18:19])

                # rgb head
                for sub in range(NSUB):
                    ptr = psum.tile([3, F], F32, name="rgbps", tag="mm")
                    nc.tensor.matmul(ptr[:], wrgb_t[:],
                                     h[:, sub * F:(sub + 1) * F],
                                     start=True, stop=True)
                    nc.scalar.activation(rgbsb[:, sub * F:(sub + 1) * F],
                                         ptr[:], AF.Sigmoid,
                                         bias=b_t[0:3, 20:21])

                nc.sync.dma_start(out[0:1, sl], osb[:])
                nc.sync.dma_start(out[1:4, sl], rgbsb[:])

    nc.compile()
    _cache[key] = nc
    return nc


def _prep_inputs(inputs):
    """Host-side shard + transpose prep. Returns list of per-core dicts."""
    f32 = np.float32
    sp = np.ascontiguousarray(inputs["sample_points"], dtype=f32)
    dirs = np.ascontiguousarray(inputs["directions"], dtype=f32)

    dirs_all = dirs.T.copy()                          # [3, 4096]

    def wt(w):  # [out, in] -> [in, out]
        return np.ascontiguousarray(w.T, dtype=f32)

    def wmid_pack(w):  # [256, 256] -> [128, 512] (k-chunk blocks)
        t = wt(w)                                     # [256, 256]
        return np.ascontiguousarray(
            t.reshape(2, 128, 256).transpose(1, 0, 2).reshape(128, 512))

    shared = {}
    shared["w0"] = wt(inputs["Wx0"])                  # [63, 256]
    for i in range(1, 8):
        w = inputs[f"Wx{i}"]
        if i == 4:
            shared["wmid4"] = wmid_pack(w[:, :256])
            shared["w4e"] = wt(w[:, 256:])            # [63, 256]
        else:
            shared[f"wmid{i}"] = wmid_pack(w)
    shared["wfeat"] = wmid_pack(inputs["Wfeat"])
    wden_t = wt(inputs["Wden"])                       # [256, 1]
    shared["wden"] = np.ascontiguousarray(
        wden_t.reshape(2, 128, 1).transpose(1, 0, 2).reshape(128, 2))
    wd0_t = wt(inputs["Wd0"])                         # [283, 128]
    shared["wd0"] = np.ascontiguousarray(
        wd0_t[:256].reshape(2, 128, 128).transpose(1, 0, 2).reshape(128, 256))
    shared["wd0e"] = np.ascontiguousarray(wd0_t[256:])  # [27, 128]
    shared["wrgb"] = wt(inputs["Wrgb"])               # [128, 3]

    bias = np.zeros((128, 21), dtype=f32)
    for li in range(8):
        b = inputs[f"bx{li}"]
        bias[:, 2 * li] = b[:128]
        bias[:, 2 * li + 1] = b[128:]
    bias[:, 16] = inputs["bfeat"][:128]
    bias[:, 17] = inputs["bfeat"][128:]
    bias[:, 18] = inputs["bd0"]
    bias[0, 19] = inputs["bden"][0]
    bias[0:3, 20] = inputs["brgb"]

    consts = np.zeros((128, 4), dtype=f32)
    consts[0:30, 0] = 2.0 ** (np.arange(30) % 10)
    consts[30:60, 0] = 2.0 ** (np.arange(30) % 10)
    consts[30:60, 1] = np.pi / 2
    consts[0:12, 2] = 2.0 ** (np.arange(12) % 4)
    consts[12:24, 2] = 2.0 ** (np.arange(12) % 4)
    consts[12:24, 3] = np.pi / 2

    in_maps = []
    for c in range(N_CORES):
        m = dict(shared)
        # sample-major: [3, S, R] flattened to [3, NPTS]
        blk = sp[c * R_CORE:(c + 1) * R_CORE]         # [R, S, 3]
        m["pts"] = np.ascontiguousarray(
            blk.transpose(2, 1, 0).reshape(3, NPTS))
        m["dirs"] = np.ascontiguousarray(
            dirs_all[:, c * R_CORE:(c + 1) * R_CORE])
        m["biases"] = bias
        m["consts"] = consts
        in_maps.append(m)
    return in_maps


def kernel(**inputs) -> np.ndarray:
    nc = _build()
    in_maps = _prep_inputs(inputs)
    res = run_bass_kernel_spmd(nc, in_maps, core_ids=list(range(N_CORES)))
    outs = []
    for c in range(N_CORES):
        o = res.results[c]["out"]                     # [4, NPTS] sample-major
        outs.append(o.reshape(4, S, R_CORE).transpose(2, 1, 0))
    return np.concatenate(outs, axis=0)
